# revision 2
# baseline (speedup 1.0000x reference)
"""CentroidAttention Trainium2 kernel.

reference:
    att[n,t] = dot(c_ft[n,t,:], ft[n,:])        (N,16,64)x(N,64) -> (N,16)
    att = softmax(att, axis=-1)
    out[n,d] = sum_t att[n,t] * c_ft[n,t,d]     -> (N,64)

Sharding: pure data parallel over N across 8 NeuronCores (no collectives).

Per-core layout ("B3"): nodes processed in tiles of 256.
  SBUF partition p = n16*8 + t_lo   (n16 in [0,16), t_lo in [0,8))
  free dims       = (g in [0,16), t_hi in [0,2), d in [0,64))
  node n = tile*256 + g*16 + n16 ;  t = t_lo*2 + t_hi
This keeps HBM reads contiguous in 512B runs (t_hi,d) and puts 8 of the
16 t's of each node on distinct partitions so the weighted t-sum can run
on the tensor engine as block-diagonal matmuls (attention weights folded
into the stationary operand; partition contraction sums t_lo, PSUM
accumulation sums t_hi).

Engines:
  DVE: P1 c_ft*ft multiply, part of d-reduce, softmax normalize, BD build
  ACT: rest of d-reduce (activation accum_out), exp, PSUM->SBUF out copy
  PE : ft broadcast 16->128 parts, softmax sum + recip broadcast, weighted t-sum
"""

import os
from contextlib import ExitStack

import numpy as np

import concourse.bass as bass
import concourse.bacc as bacc
import concourse.tile as tile
from concourse import mybir
from concourse.bass_utils import run_bass_kernel_spmd

F32 = mybir.dt.float32

N_CORES = 8
N, T, D = 200000, 16, 64
NPC = 25088            # nodes per core; 8*25088 = 200704 (704 zero-pad nodes)
TILE_N = 256           # nodes per tile
G = 16                 # node-groups of 16 per tile
N16 = 16               # nodes per partition-block
TLO, THI = 8, 2        # t = t_lo*2 + t_hi
GD = 9                 # g-columns whose d-reduce runs on DVE (rest on ACT)
EXP_BIAS = 0.0         # constant subtracted from att before exp (softmax-invariant)


def build_program(npc=NPC, gd=GD, exp_bias=EXP_BIAS):
    """Build + compile the single-core Bass program (SPMD across 8 cores)."""
    nt = npc // TILE_N
    assert npc % TILE_N == 0

    nc = bacc.Bacc("TRN2", target_bir_lowering=False, debug=False)

    c_dram = nc.dram_tensor("c_ft", [npc, T, D], F32, kind="ExternalInput")
    f_dram = nc.dram_tensor("ft", [npc, D], F32, kind="ExternalInput")
    mask_dram = nc.dram_tensor("mask16", [128, 16], F32, kind="ExternalInput")
    maskT_dram = nc.dram_tensor("m16T", [16, 128], F32, kind="ExternalInput")
    out_dram = nc.dram_tensor("out", [npc, D], F32, kind="ExternalOutput")

    # [nt, (n16 tlo)=128, g, (thi d)=128]
    c_ap = c_dram.ap().rearrange(
        "(nt g n16) (tlo thi) d -> nt (n16 tlo) g (thi d)",
        nt=nt, g=G, n16=N16, tlo=TLO, thi=THI,
    )
    # [nt, n16=16, g, d]
    f_ap = f_dram.ap().rearrange("(nt g n16) d -> nt n16 g d", nt=nt, g=G, n16=N16)
    # [nt, m=16, g, d]
    o_ap = out_dram.ap().rearrange("(nt g m) d -> nt m g d", nt=nt, g=G, m=N16)

    PS = bass.MemorySpace.PSUM
    with tile.TileContext(nc) as tc, ExitStack() as ctx:
        const_pool = ctx.enter_context(tc.tile_pool(name="const", bufs=1))
        cb_pool = ctx.enter_context(tc.tile_pool(name="cb", bufs=3))
        ftd_pool = ctx.enter_context(tc.tile_pool(name="ftd", bufs=3))
        prod_pool = ctx.enter_context(tc.tile_pool(name="prod", bufs=2))
        small_pool = ctx.enter_context(tc.tile_pool(name="small", bufs=2))
        dummy_pool = ctx.enter_context(tc.tile_pool(name="dummy", bufs=4))
        outsb_pool = ctx.enter_context(tc.tile_pool(name="outsb", bufs=2))
        ftb_pool = ctx.enter_context(tc.tile_pool(name="ftb", bufs=2, space=PS))
        sps_pool = ctx.enter_context(tc.tile_pool(name="sps", bufs=1, space=PS))
        rbps_pool = ctx.enter_context(tc.tile_pool(name="rbps", bufs=1, space=PS))
        ops_pool = ctx.enter_context(tc.tile_pool(name="ops", bufs=2, space=PS))

        mask_sb = const_pool.tile([128, 16], F32)
        nc.sync.dma_start(mask_sb[:], mask_dram.ap())
        maskT_sb = const_pool.tile([16, 128], F32)
        nc.sync.dma_start(maskT_sb[:], maskT_dram.ap())

        for ts in range(nt):
            # ---- loads ----
            cb = cb_pool.tile([128, G, THI * D], F32)          # [128,16,128]
            nc.sync.dma_start(cb[:], c_ap[ts])
            cb_v = cb[:].rearrange("p g (thi d) -> p g thi d", thi=THI)

            ftd = ftd_pool.tile([16, G, D], F32)               # [16,16,64]
            nc.sync.dma_start(ftd[:], f_ap[ts])

            # ---- ft broadcast to all 128 partitions (PE), 2 halves ----
            ftb = []
            for h in range(2):
                fb = ftb_pool.tile([128, (G // 2) * D], F32)   # [128,512] psum
                nc.tensor.matmul(
                    fb[:],
                    maskT_sb[:],
                    ftd[:, h * (G // 2):(h + 1) * (G // 2), :],
                    start=True, stop=True,
                )
                ftb.append(fb)

            # ---- P1: prod = c_ft * ft  (DVE, 2 halves) ----
            prod = prod_pool.tile([128, G, THI, D], F32)       # [128,2048]
            for h in range(2):
                in1 = (
                    ftb[h][:]
                    .rearrange("p (g d) -> p g d", g=G // 2)[:, :, None, :]
                    .broadcast_to([128, G // 2, THI, D])
                )
                g0 = h * (G // 2)
                nc.vector.tensor_mul(
                    prod[:, g0:g0 + G // 2], cb_v[:, g0:g0 + G // 2], in1
                )

            # ---- P2: att[p,(g,thi)] = sum_d prod  (DVE for g<gd, ACT rest) ----
            att = small_pool.tile([128, G * THI], F32)         # [128,32]
            att_v = att[:].rearrange("p (g thi) -> p g thi", thi=THI)
            if gd > 0:
                nc.vector.reduce_sum(
                    att_v[:, 0:gd], prod[:, 0:gd], axis=mybir.AxisListType.X
                )
            for c in range(gd * THI, G * THI):
                g, thi = divmod(c, THI)
                dmy = dummy_pool.tile([128, D], F32)
                nc.scalar.activation(
                    dmy[:], prod[:, g, thi],
                    mybir.ActivationFunctionType.Copy,
                    accum_out=att[:, c:c + 1],
                )

            # ---- softmax pieces ----
            E = small_pool.tile([128, G * THI], F32)
            nc.scalar.activation(
                E[:], att[:], mybir.ActivationFunctionType.Exp, bias=exp_bias
            )
            E_v = E[:].rearrange("p (g thi) -> p g thi", thi=THI)

            S = sps_pool.tile([16, G], F32)                    # psum
            for thi in range(THI):
                nc.tensor.matmul(
                    S[:], mask_sb[:], E_v[:, :, thi],
                    start=(thi == 0), stop=(thi == THI - 1),
                )

            rS = small_pool.tile([16, G], F32)
            nc.vector.reciprocal(rS[:], S[:])

            rb = rbps_pool.tile([128, G], F32)                 # psum
            nc.tensor.matmul(rb[:], maskT_sb[:], rS[:], start=True, stop=True)

            attn = small_pool.tile([128, G * THI], F32)
            attn_v = attn[:].rearrange("p (g thi) -> p g thi", thi=THI)
            rb_b = rb[:][:, :, None].broadcast_to([128, G, THI])
            nc.vector.tensor_mul(attn_v, E_v, rb_b)

            # ---- block-diag attention weights BD[p,(g,thi,m)] ----
            BD = small_pool.tile([128, G, THI, N16], F32)      # [128,512]
            a_b = attn_v[:, :, :, None].broadcast_to([128, G, THI, N16])
            m_b = mask_sb[:][:, None, None, :].broadcast_to([128, G, THI, N16])
            nc.vector.tensor_mul(BD[:], a_b, m_b)

            # ---- P4: out[m,(g,d)] = sum_{tlo,thi} BD^T @ (cb slice)  (PE) ----
            ops = ops_pool.tile([16, G, D], F32)               # [16,1024] psum
            for g in range(G):
                for thi in range(THI):
                    nc.tensor.matmul(
                        ops[:, g, :], BD[:, g, thi], cb_v[:, g, thi],
                        start=(thi == 0), stop=(thi == THI - 1),
                    )

            outsb = outsb_pool.tile([16, G, D], F32)
            nc.scalar.copy(outsb[:], ops[:])
            nc.sync.dma_start(o_ap[ts], outsb[:])

    nc.compile()
    return nc


def make_consts():
    p = np.arange(128)
    mask16 = (p[:, None] // 8 == np.arange(16)[None, :]).astype(np.float32)
    return mask16, np.ascontiguousarray(mask16.T)


_CACHE = {}


def _get_program(npc):
    if npc not in _CACHE:
        _CACHE[npc] = build_program(npc=npc)
    return _CACHE[npc]


def kernel(c_ft: np.ndarray, ft: np.ndarray) -> np.ndarray:
    c_ft = np.ascontiguousarray(np.asarray(c_ft), dtype=np.float32)
    ft = np.ascontiguousarray(np.asarray(ft), dtype=np.float32)
    n = c_ft.shape[0]
    total = N_CORES * NPC
    if n < total:
        c_pad = np.zeros((total, T, D), np.float32)
        c_pad[:n] = c_ft
        f_pad = np.zeros((total, D), np.float32)
        f_pad[:n] = ft
    else:
        c_pad, f_pad = c_ft, ft

    mask16, m16T = make_consts()
    nc = _get_program(NPC)
    in_maps = [
        {
            "c_ft": c_pad[i * NPC:(i + 1) * NPC],
            "ft": f_pad[i * NPC:(i + 1) * NPC],
            "mask16": mask16,
            "m16T": m16T,
        }
        for i in range(N_CORES)
    ]
    res = run_bass_kernel_spmd(
        nc, in_maps, core_ids=list(range(N_CORES)),
        trace=bool(int(os.environ.get("KERNEL_TRACE", "0"))),
    )
    out = np.concatenate([r["out"] for r in res.results], axis=0)
    if bool(int(os.environ.get("KERNEL_TRACE", "0"))):
        kernel.last_exec_time_ns = res.exec_time_ns
        kernel.last_results = res
    return out[:n]


# revision 9
# speedup vs baseline: 1.2568x; 1.2568x over previous
"""CentroidAttention Trainium2 kernel.

reference:
    att[n,t] = dot(c_ft[n,t,:], ft[n,:])        (N,16,64)x(N,64) -> (N,16)
    att = softmax(att, axis=-1)
    out[n,d] = sum_t att[n,t] * c_ft[n,t,d]     -> (N,64)

Sharding: pure data parallel over N across 8 NeuronCores (no collectives).

Per-core layout ("B3"): nodes processed in tiles of 256.
  SBUF partition p = n16*8 + t_lo   (n16 in [0,16), t_lo in [0,8))
  free dims       = (g in [0,16), t_hi in [0,2), d in [0,64))
  node n = tile*256 + g*16 + n16 ;  t = t_lo*2 + t_hi
This keeps HBM reads contiguous in 512B runs (t_hi,d) and puts 8 of the
16 t's of each node on distinct partitions so the weighted t-sum can run
on the tensor engine as block-diagonal matmuls (attention weights folded
into the stationary operand; partition contraction sums t_lo, PSUM
accumulation sums t_hi).

Engines:
  DVE: P1 c_ft*ft multiply, part of d-reduce, softmax normalize, BD build
  ACT: rest of d-reduce (activation accum_out), exp, PSUM->SBUF out copy
  PE : ft broadcast 16->128 parts, softmax sum + recip broadcast, weighted t-sum
"""

import os
from contextlib import ExitStack

import numpy as np

import concourse.bass as bass
import concourse.bacc as bacc
import concourse.tile as tile
from concourse import mybir
from concourse.bass_utils import run_bass_kernel_spmd

F32 = mybir.dt.float32

N_CORES = 8
N, T, D = 200000, 16, 64
NPC = 25088            # nodes per core; 8*25088 = 200704 (704 zero-pad nodes)
TILE_N = 256           # nodes per tile
G = 16                 # node-groups of 16 per tile
N16 = 16               # nodes per partition-block
TLO, THI = 8, 2        # t = t_lo*2 + t_hi
GD = 16                # g-columns whose d-reduce runs on DVE (rest on ACT)
EXP_BIAS = 0.0         # constant subtracted from att before exp (softmax-invariant)


def build_program(npc=NPC, gd=GD, exp_bias=EXP_BIAS):
    """Build + compile the single-core Bass program (SPMD across 8 cores)."""
    nt = npc // TILE_N
    assert npc % TILE_N == 0

    nc = bacc.Bacc("TRN2", target_bir_lowering=False, debug=False)

    c_dram = nc.dram_tensor("c_ft", [npc, T, D], F32, kind="ExternalInput")
    f_dram = nc.dram_tensor("ft", [npc, D], F32, kind="ExternalInput")
    mask_dram = nc.dram_tensor("mask16", [128, 16], F32, kind="ExternalInput")
    maskT_dram = nc.dram_tensor("m16T", [16, 128], F32, kind="ExternalInput")
    out_dram = nc.dram_tensor("out", [npc, D], F32, kind="ExternalOutput")

    # [nt, (n16 tlo)=128, g, (thi d)=128]
    c_ap = c_dram.ap().rearrange(
        "(nt g n16) (tlo thi) d -> nt (n16 tlo) g (thi d)",
        nt=nt, g=G, n16=N16, tlo=TLO, thi=THI,
    )
    # [nt, n16=16, g, d]
    f_ap = f_dram.ap().rearrange("(nt g n16) d -> nt n16 g d", nt=nt, g=G, n16=N16)
    # node n = ts*256 + g*16 + m with g = 4*gh + j  ->  [nt, j, m, gh, d]
    o_ap = out_dram.ap().rearrange(
        "(nt gh j m) d -> nt j m gh d", nt=nt, gh=G // 4, j=4, m=N16
    )

    PS = bass.MemorySpace.PSUM
    with tile.TileContext(nc) as tc, ExitStack() as ctx:
        const_pool = ctx.enter_context(tc.tile_pool(name="const", bufs=1))
        cb_pool = ctx.enter_context(tc.tile_pool(name="cb", bufs=3))
        ftd_pool = ctx.enter_context(tc.tile_pool(name="ftd", bufs=3))
        prod_pool = ctx.enter_context(tc.tile_pool(name="prod", bufs=2))
        small_pool = ctx.enter_context(tc.tile_pool(name="small", bufs=2))
        dummy_pool = ctx.enter_context(tc.tile_pool(name="dummy", bufs=4))
        outsb_pool = ctx.enter_context(tc.tile_pool(name="outsb", bufs=2))
        ftb_pool = ctx.enter_context(tc.tile_pool(name="ftb", bufs=2, space=PS))
        sps_pool = ctx.enter_context(tc.tile_pool(name="sps", bufs=2, space=PS))
        rbps_pool = ctx.enter_context(tc.tile_pool(name="rbps", bufs=2, space=PS))
        ops_pool = ctx.enter_context(tc.tile_pool(name="ops", bufs=2, space=PS))

        mask_sb = const_pool.tile([128, 16], F32)
        nc.sync.dma_start(mask_sb[:], mask_dram.ap())
        maskT_sb = const_pool.tile([16, 128], F32)
        nc.sync.dma_start(maskT_sb[:], maskT_dram.ap())

        for ts in range(nt):
            # ---- loads ----
            cb = cb_pool.tile([128, G, THI * D], F32)          # [128,16,128]
            nc.sync.dma_start(cb[:], c_ap[ts])
            cb_v = cb[:].rearrange("p g (thi d) -> p g thi d", thi=THI)

            ftd = ftd_pool.tile([16, G, D], F32)               # [16,16,64]
            nc.sync.dma_start(ftd[:], f_ap[ts])

            # ---- ft broadcast to all 128 partitions (PE), 2 halves ----
            ftb = []
            for h in range(2):
                fb = ftb_pool.tile([128, (G // 2) * D], F32)   # [128,512] psum
                nc.tensor.matmul(
                    fb[:],
                    maskT_sb[:],
                    ftd[:, h * (G // 2):(h + 1) * (G // 2), :],
                    start=True, stop=True,
                )
                ftb.append(fb)

            # ---- P1: prod = c_ft * ft  (DVE, 2 halves) ----
            prod = prod_pool.tile([128, G, THI, D], F32)       # [128,2048]
            for h in range(2):
                in1 = (
                    ftb[h][:]
                    .rearrange("p (g d) -> p g d", g=G // 2)[:, :, None, :]
                    .broadcast_to([128, G // 2, THI, D])
                )
                g0 = h * (G // 2)
                nc.vector.tensor_mul(
                    prod[:, g0:g0 + G // 2], cb_v[:, g0:g0 + G // 2], in1
                )

            # ---- P2: att[p,(g,thi)] = sum_d prod  (DVE for g<gd, ACT rest) ----
            att = small_pool.tile([128, G * THI], F32)         # [128,32]
            att_v = att[:].rearrange("p (g thi) -> p g thi", thi=THI)
            if gd > 0:
                nc.vector.reduce_sum(
                    att_v[:, 0:gd], prod[:, 0:gd], axis=mybir.AxisListType.X
                )
            for c in range(gd * THI, G * THI):
                g, thi = divmod(c, THI)
                dmy = dummy_pool.tile([128, D], F32)
                nc.scalar.activation(
                    dmy[:], prod[:, g, thi],
                    mybir.ActivationFunctionType.Copy,
                    accum_out=att[:, c:c + 1],
                )

            # ---- softmax pieces ----
            E = small_pool.tile([128, G * THI], F32)
            nc.scalar.activation(
                E[:], att[:], mybir.ActivationFunctionType.Exp, bias=exp_bias
            )
            E_v = E[:].rearrange("p (g thi) -> p g thi", thi=THI)

            S = sps_pool.tile([16, G], F32)                    # psum
            for thi in range(THI):
                nc.tensor.matmul(
                    S[:], mask_sb[:], E_v[:, :, thi],
                    start=(thi == 0), stop=(thi == THI - 1),
                )

            rS = small_pool.tile([16, G], F32)
            nc.vector.reciprocal(rS[:], S[:])

            rb = rbps_pool.tile([128, G], F32)                 # psum
            nc.tensor.matmul(rb[:], maskT_sb[:], rS[:], start=True, stop=True)

            attn = small_pool.tile([128, G * THI], F32)
            attn_v = attn[:].rearrange("p (g thi) -> p g thi", thi=THI)
            rb_b = rb[:][:, :, None].broadcast_to([128, G, THI])
            nc.vector.tensor_mul(attn_v, E_v, rb_b)

            # ---- block-diag attention weights BD[p,(g,thi,m)] ----
            BD = small_pool.tile([128, G, THI, N16], F32)      # [128,512]
            a_b = attn_v[:, :, :, None].broadcast_to([128, G, THI, N16])
            m_b = mask_sb[:][:, None, None, :].broadcast_to([128, G, THI, N16])
            nc.vector.tensor_mul(BD[:], a_b, m_b)

            # ---- P4: out = BD^T @ (cb slice), 4 groups col-tiled across PE
            # quadrants.  group g = 4*gh + j -> PSUM partitions [32j,32j+16),
            # free cols [64*gh, 64*gh+64).  Accumulate over thi. ----
            # PSUM start=True zeroes the whole 2KB bank in the written
            # partitions, so each partition-quad j starts exactly once (its
            # first matmul) and accumulates through all gh/thi phases.
            ops = ops_pool.tile([128, (G // 4) * D], F32)      # [128,256] psum
            ops_v = ops[:].rearrange("p (gh d) -> p gh d", gh=G // 4)
            for qb in range(0, G, 4):
                gh = qb // 4
                for thi in range(THI):
                    for j in range(4):
                        g = qb + j
                        nc.tensor.matmul(
                            ops_v[32 * j:32 * j + 16, gh],
                            BD[:, g, thi], cb_v[:, g, thi],
                            start=(qb == 0 and thi == 0),
                            stop=(qb == G - 4 and thi == THI - 1),
                            tile_position=(0, 32 * j),
                            skip_group_check=True,
                        )

            outsb = outsb_pool.tile([128, (G // 4) * D], F32)
            for j in range(4):
                nc.scalar.copy(
                    outsb[32 * j:32 * j + 16], ops[32 * j:32 * j + 16]
                )
                src = outsb[32 * j:32 * j + 16].rearrange(
                    "m (gh d) -> m gh d", gh=G // 4
                )
                nc.sync.dma_start(o_ap[ts][j], src)

    nc.compile()
    return nc


def make_consts():
    p = np.arange(128)
    mask16 = (p[:, None] // 8 == np.arange(16)[None, :]).astype(np.float32)
    return mask16, np.ascontiguousarray(mask16.T)


_CACHE = {}


def _get_program(npc):
    if npc not in _CACHE:
        _CACHE[npc] = build_program(npc=npc)
    return _CACHE[npc]


def kernel(c_ft: np.ndarray, ft: np.ndarray) -> np.ndarray:
    c_ft = np.ascontiguousarray(np.asarray(c_ft), dtype=np.float32)
    ft = np.ascontiguousarray(np.asarray(ft), dtype=np.float32)
    n = c_ft.shape[0]
    total = N_CORES * NPC
    if n < total:
        c_pad = np.zeros((total, T, D), np.float32)
        c_pad[:n] = c_ft
        f_pad = np.zeros((total, D), np.float32)
        f_pad[:n] = ft
    else:
        c_pad, f_pad = c_ft, ft

    mask16, m16T = make_consts()
    nc = _get_program(NPC)
    in_maps = [
        {
            "c_ft": c_pad[i * NPC:(i + 1) * NPC],
            "ft": f_pad[i * NPC:(i + 1) * NPC],
            "mask16": mask16,
            "m16T": m16T,
        }
        for i in range(N_CORES)
    ]
    res = run_bass_kernel_spmd(
        nc, in_maps, core_ids=list(range(N_CORES)),
        trace=bool(int(os.environ.get("KERNEL_TRACE", "0"))),
    )
    out = np.concatenate([r["out"] for r in res.results], axis=0)
    if bool(int(os.environ.get("KERNEL_TRACE", "0"))):
        kernel.last_exec_time_ns = res.exec_time_ns
        kernel.last_results = res
    return out[:n]


# revision 13
# speedup vs baseline: 1.3435x; 1.0689x over previous
"""CentroidAttention Trainium2 kernel.

reference:
    att[n,t] = dot(c_ft[n,t,:], ft[n,:])        (N,16,64)x(N,64) -> (N,16)
    att = softmax(att, axis=-1)
    out[n,d] = sum_t att[n,t] * c_ft[n,t,d]     -> (N,64)

Sharding: pure data parallel over N across 8 NeuronCores (no collectives).

Per-core layout ("B3"): nodes processed in tiles of 256.
  SBUF partition p = n16*8 + t_lo   (n16 in [0,16), t_lo in [0,8))
  free dims       = (g in [0,16), t_hi in [0,2), d in [0,64))
  node n = tile*256 + g*16 + n16 ;  t = t_lo*2 + t_hi
This keeps HBM reads contiguous in 512B runs (t_hi,d) and puts 8 of the
16 t's of each node on distinct partitions so the weighted t-sum can run
on the tensor engine as block-diagonal matmuls (attention weights folded
into the stationary operand; partition contraction sums t_lo, PSUM
accumulation sums t_hi).

Engines:
  DVE: P1 c_ft*ft multiply, part of d-reduce, softmax normalize, BD build
  ACT: rest of d-reduce (activation accum_out), exp, PSUM->SBUF out copy
  PE : ft broadcast 16->128 parts, softmax sum + recip broadcast, weighted t-sum
"""

import os
from contextlib import ExitStack

import numpy as np

import concourse.bass as bass
import concourse.bacc as bacc
import concourse.tile as tile
from concourse import mybir
from concourse.bass_utils import run_bass_kernel_spmd

F32 = mybir.dt.float32

N_CORES = 8
N, T, D = 200000, 16, 64
NPC = 25088            # nodes per core; 8*25088 = 200704 (704 zero-pad nodes)
TILE_N = 256           # nodes per tile
G = 16                 # node-groups of 16 per tile
N16 = 16               # nodes per partition-block
TLO, THI = 8, 2        # t = t_lo*2 + t_hi
GD = 16                # g-columns whose d-reduce runs on DVE (rest on ACT)
EXP_BIAS = 0.0         # constant subtracted from att before exp (softmax-invariant)


def build_program(npc=NPC, gd=GD, exp_bias=EXP_BIAS):
    """Build + compile the single-core Bass program (SPMD across 8 cores)."""
    nt = npc // TILE_N
    assert npc % TILE_N == 0

    nc = bacc.Bacc("TRN2", target_bir_lowering=False, debug=False)

    c_dram = nc.dram_tensor("c_ft", [npc, T, D], F32, kind="ExternalInput")
    f_dram = nc.dram_tensor("ft", [npc, D], F32, kind="ExternalInput")
    mask_dram = nc.dram_tensor("mask16", [128, 16], F32, kind="ExternalInput")
    maskT_dram = nc.dram_tensor("m16T", [16, 128], F32, kind="ExternalInput")
    out_dram = nc.dram_tensor("out", [npc, D], F32, kind="ExternalOutput")

    # [nt, (n16 tlo)=128, g, (thi d)=128]
    c_ap = c_dram.ap().rearrange(
        "(nt g n16) (tlo thi) d -> nt (n16 tlo) g (thi d)",
        nt=nt, g=G, n16=N16, tlo=TLO, thi=THI,
    )
    # [nt, n16=16, g, d]
    f_ap = f_dram.ap().rearrange("(nt g n16) d -> nt n16 g d", nt=nt, g=G, n16=N16)
    # node n = ts*256 + g*16 + m with g = 4*gh + j  ->  [nt, j, m, gh, d]
    o_ap = out_dram.ap().rearrange(
        "(nt gh j m) d -> nt j m gh d", nt=nt, gh=G // 4, j=4, m=N16
    )

    PS = bass.MemorySpace.PSUM
    with tile.TileContext(nc) as tc, ExitStack() as ctx:
        const_pool = ctx.enter_context(tc.tile_pool(name="const", bufs=1))
        cb_pool = ctx.enter_context(tc.tile_pool(name="cb", bufs=4))
        ftd_pool = ctx.enter_context(tc.tile_pool(name="ftd", bufs=4))
        prod_pool = ctx.enter_context(tc.tile_pool(name="prod", bufs=3))
        small_pool = ctx.enter_context(tc.tile_pool(name="small", bufs=3))
        dummy_pool = ctx.enter_context(tc.tile_pool(name="dummy", bufs=4))
        outsb_pool = ctx.enter_context(tc.tile_pool(name="outsb", bufs=3))
        ftb_pool = ctx.enter_context(tc.tile_pool(name="ftb", bufs=4, space=PS))
        sps_pool = ctx.enter_context(tc.tile_pool(name="sps", bufs=1, space=PS))
        rbps_pool = ctx.enter_context(tc.tile_pool(name="rbps", bufs=1, space=PS))
        ops_pool = ctx.enter_context(tc.tile_pool(name="ops", bufs=2, space=PS))

        mask_sb = const_pool.tile([128, 16], F32)
        nc.sync.dma_start(mask_sb[:], mask_dram.ap())
        maskT_sb = const_pool.tile([16, 128], F32)
        nc.sync.dma_start(maskT_sb[:], maskT_dram.ap())

        for ts in range(nt):
            # ---- loads ----
            cb = cb_pool.tile([128, G, THI * D], F32)          # [128,16,128]
            nc.sync.dma_start(cb[:], c_ap[ts])
            cb_v = cb[:].rearrange("p g (thi d) -> p g thi d", thi=THI)

            ftd = ftd_pool.tile([16, G, D], F32)               # [16,16,64]
            nc.sync.dma_start(ftd[:], f_ap[ts])

            # ---- ft broadcast to all 128 partitions (PE), 2 halves ----
            ftb = []
            for h in range(2):
                fb = ftb_pool.tile([128, (G // 2) * D], F32)   # [128,512] psum
                nc.tensor.matmul(
                    fb[:],
                    maskT_sb[:],
                    ftd[:, h * (G // 2):(h + 1) * (G // 2), :],
                    start=True, stop=True,
                )
                ftb.append(fb)

            # ---- P1: prod = c_ft * ft  (DVE, 2 halves) ----
            prod = prod_pool.tile([128, G, THI, D], F32)       # [128,2048]
            for h in range(2):
                in1 = (
                    ftb[h][:]
                    .rearrange("p (g d) -> p g d", g=G // 2)[:, :, None, :]
                    .broadcast_to([128, G // 2, THI, D])
                )
                g0 = h * (G // 2)
                nc.vector.tensor_mul(
                    prod[:, g0:g0 + G // 2], cb_v[:, g0:g0 + G // 2], in1
                )

            # ---- P2: att[p,(g,thi)] = sum_d prod  (DVE for g<gd, ACT rest) ----
            att = small_pool.tile([128, G * THI], F32)         # [128,32]
            att_v = att[:].rearrange("p (g thi) -> p g thi", thi=THI)
            if gd > 0:
                nc.vector.reduce_sum(
                    att_v[:, 0:gd], prod[:, 0:gd], axis=mybir.AxisListType.X
                )
            for c in range(gd * THI, G * THI):
                g, thi = divmod(c, THI)
                dmy = dummy_pool.tile([128, D], F32)
                nc.scalar.activation(
                    dmy[:], prod[:, g, thi],
                    mybir.ActivationFunctionType.Copy,
                    accum_out=att[:, c:c + 1],
                )

            # ---- softmax pieces ----
            E = small_pool.tile([128, G * THI], F32)
            nc.scalar.activation(
                E[:], att[:], mybir.ActivationFunctionType.Exp, bias=exp_bias
            )
            E_v = E[:].rearrange("p (g thi) -> p g thi", thi=THI)

            S = sps_pool.tile([16, G], F32)                    # psum
            for thi in range(THI):
                nc.tensor.matmul(
                    S[:], mask_sb[:], E_v[:, :, thi],
                    start=(thi == 0), stop=(thi == THI - 1),
                )

            rS = small_pool.tile([16, G], F32)
            nc.vector.reciprocal(rS[:], S[:])

            rb = rbps_pool.tile([128, G], F32)                 # psum
            nc.tensor.matmul(rb[:], maskT_sb[:], rS[:], start=True, stop=True)

            attn = small_pool.tile([128, G * THI], F32)
            attn_v = attn[:].rearrange("p (g thi) -> p g thi", thi=THI)
            rb_b = rb[:][:, :, None].broadcast_to([128, G, THI])
            nc.vector.tensor_mul(attn_v, E_v, rb_b)

            # ---- block-diag attention weights BD[p,(g,thi,m)] ----
            BD = small_pool.tile([128, G, THI, N16], F32)      # [128,512]
            a_b = attn_v[:, :, :, None].broadcast_to([128, G, THI, N16])
            m_b = mask_sb[:][:, None, None, :].broadcast_to([128, G, THI, N16])
            nc.gpsimd.tensor_mul(BD[:], a_b, m_b)

            # ---- P4: out = BD^T @ (cb slice), 4 groups col-tiled across PE
            # quadrants.  group g = 4*gh + j -> PSUM partitions [32j,32j+16),
            # free cols [64*gh, 64*gh+64).  Accumulate over thi. ----
            # PSUM start=True zeroes the whole 2KB bank in the written
            # partitions, so each partition-quad j starts exactly once (its
            # first matmul) and accumulates through all gh/thi phases.
            ops = ops_pool.tile([128, (G // 4) * D], F32)      # [128,256] psum
            ops_v = ops[:].rearrange("p (gh d) -> p gh d", gh=G // 4)
            for qb in range(0, G, 4):
                gh = qb // 4
                for thi in range(THI):
                    for j in range(4):
                        g = qb + j
                        nc.tensor.matmul(
                            ops_v[32 * j:32 * j + 16, gh],
                            BD[:, g, thi], cb_v[:, g, thi],
                            start=(qb == 0 and thi == 0),
                            stop=(qb == G - 4 and thi == THI - 1),
                            tile_position=(0, 32 * j),
                            skip_group_check=True,
                        )

            outsb = outsb_pool.tile([128, (G // 4) * D], F32)
            for j in range(4):
                nc.scalar.copy(
                    outsb[32 * j:32 * j + 16], ops[32 * j:32 * j + 16]
                )
                src = outsb[32 * j:32 * j + 16].rearrange(
                    "m (gh d) -> m gh d", gh=G // 4
                )
                dma_eng = nc.sync if j < 2 else nc.scalar
                dma_eng.dma_start(o_ap[ts][j], src)

    nc.compile()
    return nc


def make_consts():
    p = np.arange(128)
    mask16 = (p[:, None] // 8 == np.arange(16)[None, :]).astype(np.float32)
    return mask16, np.ascontiguousarray(mask16.T)


_CACHE = {}


def _get_program(npc):
    if npc not in _CACHE:
        _CACHE[npc] = build_program(npc=npc)
    return _CACHE[npc]


def kernel(c_ft: np.ndarray, ft: np.ndarray) -> np.ndarray:
    c_ft = np.ascontiguousarray(np.asarray(c_ft), dtype=np.float32)
    ft = np.ascontiguousarray(np.asarray(ft), dtype=np.float32)
    n = c_ft.shape[0]
    total = N_CORES * NPC
    if n < total:
        c_pad = np.zeros((total, T, D), np.float32)
        c_pad[:n] = c_ft
        f_pad = np.zeros((total, D), np.float32)
        f_pad[:n] = ft
    else:
        c_pad, f_pad = c_ft, ft

    mask16, m16T = make_consts()
    nc = _get_program(NPC)
    in_maps = [
        {
            "c_ft": c_pad[i * NPC:(i + 1) * NPC],
            "ft": f_pad[i * NPC:(i + 1) * NPC],
            "mask16": mask16,
            "m16T": m16T,
        }
        for i in range(N_CORES)
    ]
    res = run_bass_kernel_spmd(
        nc, in_maps, core_ids=list(range(N_CORES)),
        trace=bool(int(os.environ.get("KERNEL_TRACE", "0"))),
    )
    out = np.concatenate([r["out"] for r in res.results], axis=0)
    if bool(int(os.environ.get("KERNEL_TRACE", "0"))):
        kernel.last_exec_time_ns = res.exec_time_ns
        kernel.last_results = res
    return out[:n]


# revision 14
# speedup vs baseline: 2.0747x; 1.5443x over previous
"""CentroidAttention Trainium2 kernel.

reference:
    att[n,t] = dot(c_ft[n,t,:], ft[n,:])        (N,16,64)x(N,64) -> (N,16)
    att = softmax(att, axis=-1)
    out[n,d] = sum_t att[n,t] * c_ft[n,t,d]     -> (N,64)

Sharding: pure data parallel over N across 8 NeuronCores (no collectives).

Per-core layout ("v3"): nodes processed in tiles of 256.
  SBUF partition p = n32*4 + t_lo   (n32 in [0,32), t_lo in [0,4))
  free dims       = (g in [0,8), t_hi in [0,4), d in [0,64))
  node n = tile*256 + g*32 + n32 ;  t = t_lo*4 + t_hi
HBM reads stay contiguous in 1KB runs (t_hi,d).  4 of the 16 t's of each
node sit on distinct partitions, so the weighted t-sum runs on the tensor
engine as block-diagonal matmuls (attention folded into the stationary
operand, m=32 nodes per col-quadrant via tile_position; partition
contraction sums t_lo, PSUM accumulation sums t_hi).

Engines:
  DVE   : P1 c_ft*ft multiply, d-reduce (P2), softmax normalize
  GpSimd: block-diag weight build (BD)
  ACT   : exp, PSUM->SBUF out copy, out DMA issue
  PE    : ft broadcast 32->128 parts, softmax sum + recip broadcast,
          weighted t-sum (P4)
"""

import os
from contextlib import ExitStack

import numpy as np

import concourse.bass as bass
import concourse.bacc as bacc
import concourse.tile as tile
from concourse import mybir
from concourse.bass_utils import run_bass_kernel_spmd

F32 = mybir.dt.float32

N_CORES = 8
N, T, D = 200000, 16, 64
NPC = 25088            # nodes per core; 8*25088 = 200704 (704 zero-pad nodes)
TILE_N = 256           # nodes per tile
G = 8                  # node-groups of 32 per tile
N32 = 32               # nodes per partition-block
TLO, THI = 4, 4        # t = t_lo*4 + t_hi
EXP_BIAS = 0.0


def build_program(npc=NPC, exp_bias=EXP_BIAS):
    """Build + compile the single-core Bass program (SPMD across 8 cores)."""
    nt = npc // TILE_N
    assert npc % TILE_N == 0

    nc = bacc.Bacc("TRN2", target_bir_lowering=False, debug=False)

    c_dram = nc.dram_tensor("c_ft", [npc, T, D], F32, kind="ExternalInput")
    f_dram = nc.dram_tensor("ft", [npc, D], F32, kind="ExternalInput")
    mask_dram = nc.dram_tensor("mask32", [128, N32], F32, kind="ExternalInput")
    maskT_dram = nc.dram_tensor("m32T", [N32, 128], F32, kind="ExternalInput")
    out_dram = nc.dram_tensor("out", [npc, D], F32, kind="ExternalOutput")

    # [nt, (n32 tlo)=128, g, (thi d)=256]
    c_ap = c_dram.ap().rearrange(
        "(nt g n32) (tlo thi) d -> nt (n32 tlo) g (thi d)",
        nt=nt, g=G, n32=N32, tlo=TLO, thi=THI,
    )
    # [nt, n32=32, g, d]
    f_ap = f_dram.ap().rearrange("(nt g n32) d -> nt n32 g d", nt=nt, g=G, n32=N32)
    # node n = ts*256 + g*32 + m with g = 4*gh + j  ->  [nt, (j m)=128, gh, d]
    o_ap = out_dram.ap().rearrange(
        "(nt gh j m) d -> nt (j m) gh d", nt=nt, gh=G // 4, j=4, m=N32
    )

    PS = bass.MemorySpace.PSUM
    with tile.TileContext(nc) as tc, ExitStack() as ctx:
        const_pool = ctx.enter_context(tc.tile_pool(name="const", bufs=1))
        cb_pool = ctx.enter_context(tc.tile_pool(name="cb", bufs=4))
        ftd_pool = ctx.enter_context(tc.tile_pool(name="ftd", bufs=4))
        prod_pool = ctx.enter_context(tc.tile_pool(name="prod", bufs=3))
        small_pool = ctx.enter_context(tc.tile_pool(name="small", bufs=3))
        outsb_pool = ctx.enter_context(tc.tile_pool(name="outsb", bufs=3))
        ftb_pool = ctx.enter_context(tc.tile_pool(name="ftb", bufs=2, space=PS))
        sps_pool = ctx.enter_context(tc.tile_pool(name="sps", bufs=2, space=PS))
        rbps_pool = ctx.enter_context(tc.tile_pool(name="rbps", bufs=2, space=PS))
        ops_pool = ctx.enter_context(tc.tile_pool(name="ops", bufs=2, space=PS))

        mask_sb = const_pool.tile([128, N32], F32)
        nc.sync.dma_start(mask_sb[:], mask_dram.ap())
        maskT_sb = const_pool.tile([N32, 128], F32)
        nc.sync.dma_start(maskT_sb[:], maskT_dram.ap())

        for ts in range(nt):
            # ---- loads ----
            cb = cb_pool.tile([128, G, THI * D], F32)          # [128,8,256]
            nc.sync.dma_start(cb[:], c_ap[ts])
            cb_v = cb[:].rearrange("p g (thi d) -> p g thi d", thi=THI)

            ftd = ftd_pool.tile([N32, G, D], F32)              # [32,8,64]
            nc.sync.dma_start(ftd[:], f_ap[ts])

            # ---- ft broadcast to all 128 partitions (PE) ----
            ftb = ftb_pool.tile([128, G * D], F32)             # [128,512] psum
            nc.tensor.matmul(ftb[:], maskT_sb[:], ftd[:], start=True, stop=True)
            ftb_v = ftb[:].rearrange("p (g d) -> p g d", g=G)

            # ---- P1: prod = c_ft * ft  (DVE, 2 halves) ----
            prod = prod_pool.tile([128, G, THI, D], F32)       # [128,2048]
            for h in range(2):
                g0 = h * (G // 2)
                in1 = (
                    ftb_v[:, g0:g0 + G // 2][:, :, None, :]
                    .broadcast_to([128, G // 2, THI, D])
                )
                nc.vector.tensor_mul(
                    prod[:, g0:g0 + G // 2], cb_v[:, g0:g0 + G // 2], in1
                )

            # ---- P2: att[p,(g,thi)] = sum_d prod  (DVE) ----
            att = small_pool.tile([128, G * THI], F32)         # [128,32]
            att_v = att[:].rearrange("p (g thi) -> p g thi", thi=THI)
            nc.vector.reduce_sum(att_v, prod[:], axis=mybir.AxisListType.X)

            # ---- softmax pieces ----
            E = small_pool.tile([128, G * THI], F32)
            nc.scalar.activation(
                E[:], att[:], mybir.ActivationFunctionType.Exp, bias=exp_bias
            )
            E_v = E[:].rearrange("p (g thi) -> p g thi", thi=THI)

            S = sps_pool.tile([N32, G], F32)                   # psum
            for thi in range(THI):
                nc.tensor.matmul(
                    S[:], mask_sb[:], E_v[:, :, thi],
                    start=(thi == 0), stop=(thi == THI - 1),
                )

            rS = small_pool.tile([N32, G], F32)
            nc.vector.reciprocal(rS[:], S[:])

            rb = rbps_pool.tile([128, G], F32)                 # psum
            nc.tensor.matmul(rb[:], maskT_sb[:], rS[:], start=True, stop=True)

            attn = small_pool.tile([128, G * THI], F32)
            attn_v = attn[:].rearrange("p (g thi) -> p g thi", thi=THI)
            rb_b = rb[:][:, :, None].broadcast_to([128, G, THI])
            nc.vector.tensor_mul(attn_v, E_v, rb_b)

            # ---- block-diag attention weights BD[p,(g,thi,m)] (GpSimd) ----
            BD = small_pool.tile([128, G, THI, N32], F32)      # [128,1024]
            a_b = attn_v[:, :, :, None].broadcast_to([128, G, THI, N32])
            m_b = mask_sb[:][:, None, None, :].broadcast_to([128, G, THI, N32])
            nc.gpsimd.tensor_mul(BD[:], a_b, m_b)

            # ---- P4: out = BD^T @ (cb slice), 4 groups col-tiled across PE
            # quadrants.  group g = 4*gh + j -> PSUM partitions [32j,32j+32),
            # free cols [64*gh, 64*gh+64).  One PSUM start per quadrant (the
            # start zeroes the whole 2KB bank in the written partitions);
            # everything else accumulates. ----
            ops = ops_pool.tile([128, (G // 4) * D], F32)      # [128,128] psum
            ops_v = ops[:].rearrange("p (gh d) -> p gh d", gh=G // 4)
            for thi in range(THI):
                for qb in range(0, G, 4):
                    gh = qb // 4
                    for j in range(4):
                        g = qb + j
                        nc.tensor.matmul(
                            ops_v[32 * j:32 * j + 32, gh],
                            BD[:, g, thi], cb_v[:, g, thi],
                            start=(thi == 0 and qb == 0),
                            stop=(thi == THI - 1 and qb == G - 4),
                            tile_position=(0, 32 * j),
                            skip_group_check=True,
                        )

            outsb = outsb_pool.tile([128, (G // 4) * D], F32)  # [128,128]
            nc.scalar.copy(outsb[:], ops[:])
            src = outsb[:].rearrange("p (gh d) -> p gh d", gh=G // 4)
            nc.scalar.dma_start(o_ap[ts], src)

    nc.compile()
    return nc


def make_consts():
    p = np.arange(128)
    mask = (p[:, None] // TLO == np.arange(N32)[None, :]).astype(np.float32)
    return mask, np.ascontiguousarray(mask.T)


_CACHE = {}


def _get_program(npc):
    if npc not in _CACHE:
        _CACHE[npc] = build_program(npc=npc)
    return _CACHE[npc]


def kernel(c_ft: np.ndarray, ft: np.ndarray) -> np.ndarray:
    c_ft = np.ascontiguousarray(np.asarray(c_ft), dtype=np.float32)
    ft = np.ascontiguousarray(np.asarray(ft), dtype=np.float32)
    n = c_ft.shape[0]
    total = N_CORES * NPC
    if n < total:
        c_pad = np.zeros((total, T, D), np.float32)
        c_pad[:n] = c_ft
        f_pad = np.zeros((total, D), np.float32)
        f_pad[:n] = ft
    else:
        c_pad, f_pad = c_ft, ft

    mask, maskT = make_consts()
    nc = _get_program(NPC)
    in_maps = [
        {
            "c_ft": c_pad[i * NPC:(i + 1) * NPC],
            "ft": f_pad[i * NPC:(i + 1) * NPC],
            "mask32": mask,
            "m32T": maskT,
        }
        for i in range(N_CORES)
    ]
    res = run_bass_kernel_spmd(
        nc, in_maps, core_ids=list(range(N_CORES)),
        trace=bool(int(os.environ.get("KERNEL_TRACE", "0"))),
    )
    out = np.concatenate([r["out"] for r in res.results], axis=0)
    if bool(int(os.environ.get("KERNEL_TRACE", "0"))):
        kernel.last_exec_time_ns = res.exec_time_ns
        kernel.last_results = res
    return out[:n]


# revision 22
# speedup vs baseline: 2.0923x; 1.0085x over previous
"""CentroidAttention Trainium2 kernel.

reference:
    att[n,t] = dot(c_ft[n,t,:], ft[n,:])        (N,16,64)x(N,64) -> (N,16)
    att = softmax(att, axis=-1)
    out[n,d] = sum_t att[n,t] * c_ft[n,t,d]     -> (N,64)

Sharding: pure data parallel over N across 8 NeuronCores (no collectives).

Per-core layout: nodes processed in tiles of 256.
  SBUF partition p = n32*4 + t_lo   (n32 in [0,32), t_lo in [0,4))
  free dims       = (g in [0,8), t_hi in [0,4), d in [0,64))
  node n = tile*256 + g*32 + n32 ;  t = t_lo*4 + t_hi
HBM reads stay contiguous in 1KB runs (t_hi,d).  4 of the 16 t's of each
node sit on distinct partitions, so the weighted t-sum runs on the tensor
engine as block-diagonal matmuls (attention folded into the stationary
operand, m=32 nodes per col-quadrant via tile_position; partition
contraction sums t_lo, PSUM accumulation sums t_hi).

Engines:
  DVE   : P1 c_ft*ft multiply, d-reduce (P2), softmax normalize
  GpSimd: attention normalize + block-diag weight build (BD)
  ACT   : exp, PSUM->SBUF out copy, out DMA issue
  PE    : ft broadcast 32->128 parts, softmax sum + recip broadcast,
          weighted t-sum (P4)
"""

import os
from contextlib import ExitStack

import numpy as np

import concourse.bass as bass
import concourse.bacc as bacc
import concourse.tile as tile
from concourse import mybir
from concourse.bass_utils import run_bass_kernel_spmd

F32 = mybir.dt.float32

N_CORES = 8
N, T, D = 200000, 16, 64
NPC = 25088            # nodes per core; 8*25088 = 200704 (704 zero-pad nodes)
TILE_N = 256           # nodes per tile
G = 8                  # node-groups of 32 per tile
N32 = 32               # nodes per partition-block
TLO, THI = 4, 4        # t = t_lo*4 + t_hi
EXP_BIAS = 0.0


def build_program(npc=NPC, exp_bias=EXP_BIAS):
    """Build + compile the single-core Bass program (SPMD across 8 cores)."""
    nt = npc // TILE_N
    assert npc % TILE_N == 0

    nc = bacc.Bacc("TRN2", target_bir_lowering=False, debug=False)

    c_dram = nc.dram_tensor("c_ft", [npc, T, D], F32, kind="ExternalInput")
    f_dram = nc.dram_tensor("ft", [npc, D], F32, kind="ExternalInput")
    mask_dram = nc.dram_tensor("mask32", [128, N32], F32, kind="ExternalInput")
    maskT_dram = nc.dram_tensor("m32T", [N32, 128], F32, kind="ExternalInput")
    out_dram = nc.dram_tensor("out", [npc, D], F32, kind="ExternalOutput")

    # [nt, (n32 tlo)=128, g, (thi d)=256]
    c_ap = c_dram.ap().rearrange(
        "(nt g n32) (tlo thi) d -> nt (n32 tlo) g (thi d)",
        nt=nt, g=G, n32=N32, tlo=TLO, thi=THI,
    )
    # [nt, n32=32, g, d]
    f_ap = f_dram.ap().rearrange("(nt g n32) d -> nt n32 g d", nt=nt, g=G, n32=N32)
    # node n = ts*256 + g*32 + m with g = 4*gh + j  ->  [nt, (j m)=128, gh, d]
    o_ap = out_dram.ap().rearrange(
        "(nt gh j m) d -> nt (j m) gh d", nt=nt, gh=G // 4, j=4, m=N32
    )

    PS = bass.MemorySpace.PSUM
    with tile.TileContext(nc) as tc, ExitStack() as ctx:
        const_pool = ctx.enter_context(tc.tile_pool(name="const", bufs=1))
        cb_pool = ctx.enter_context(tc.tile_pool(name="cb", bufs=4))
        prod_pool = ctx.enter_context(tc.tile_pool(name="prod", bufs=3))
        small_pool = ctx.enter_context(tc.tile_pool(name="small", bufs=3))
        outsb_pool = ctx.enter_context(tc.tile_pool(name="outsb", bufs=3))
        ftd_pool = ctx.enter_context(tc.tile_pool(name="ftd", bufs=4))
        ftb_pool = ctx.enter_context(tc.tile_pool(name="ftb", bufs=2, space=PS))
        sps_pool = ctx.enter_context(tc.tile_pool(name="sps", bufs=2, space=PS))
        rbps_pool = ctx.enter_context(tc.tile_pool(name="rbps", bufs=2, space=PS))
        ops_pool = ctx.enter_context(tc.tile_pool(name="ops", bufs=2, space=PS))

        mask_sb = const_pool.tile([128, N32], F32)
        nc.sync.dma_start(mask_sb[:], mask_dram.ap())
        maskT_sb = const_pool.tile([N32, 128], F32)
        nc.sync.dma_start(maskT_sb[:], maskT_dram.ap())

        for ts in range(nt):
            # ---- loads ----
            cb = cb_pool.tile([128, G, THI * D], F32)          # [128,8,256]
            nc.sync.dma_start(cb[:], c_ap[ts])
            cb_v = cb[:].rearrange("p g (thi d) -> p g thi d", thi=THI)

            ftd = ftd_pool.tile([N32, G, D], F32)              # [32,8,64]
            nc.sync.dma_start(ftd[:], f_ap[ts])

            # ---- ft broadcast to all 128 partitions (PE) ----
            ftb = ftb_pool.tile([128, G * D], F32)             # [128,512] psum
            nc.tensor.matmul(ftb[:], maskT_sb[:], ftd[:], start=True, stop=True)
            ftb_v = ftb[:].rearrange("p (g d) -> p g d", g=G)

            # ---- P1: prod = c_ft * ft  (DVE, 2 halves) ----
            prod = prod_pool.tile([128, G, THI, D], F32)       # [128,2048]
            for h in range(2):
                g0 = h * (G // 2)
                in1 = (
                    ftb_v[:, g0:g0 + G // 2][:, :, None, :]
                    .broadcast_to([128, G // 2, THI, D])
                )
                nc.vector.tensor_mul(
                    prod[:, g0:g0 + G // 2], cb_v[:, g0:g0 + G // 2], in1
                )

            # ---- P2: att[p,(g,thi)] = sum_d prod  (DVE) ----
            att = small_pool.tile([128, G * THI], F32)         # [128,32]
            att_v = att[:].rearrange("p (g thi) -> p g thi", thi=THI)
            nc.vector.reduce_sum(att_v, prod[:], axis=mybir.AxisListType.X)

            # ---- softmax pieces ----
            E = small_pool.tile([128, G * THI], F32)
            nc.scalar.activation(
                E[:], att[:], mybir.ActivationFunctionType.Exp, bias=exp_bias
            )
            E_v = E[:].rearrange("p (g thi) -> p g thi", thi=THI)

            S = sps_pool.tile([N32, G], F32)                   # psum
            for thi in range(THI):
                nc.tensor.matmul(
                    S[:], mask_sb[:], E_v[:, :, thi],
                    start=(thi == 0), stop=(thi == THI - 1),
                )

            rS = small_pool.tile([N32, G], F32)
            nc.vector.reciprocal(rS[:], S[:])

            rb = rbps_pool.tile([128, G], F32)                 # psum
            nc.tensor.matmul(rb[:], maskT_sb[:], rS[:], start=True, stop=True)

            attn = small_pool.tile([128, G * THI], F32)
            attn_v = attn[:].rearrange("p (g thi) -> p g thi", thi=THI)
            rb_b = rb[:][:, :, None].broadcast_to([128, G, THI])
            nc.vector.tensor_mul(attn_v, E_v, rb_b)

            # ---- block-diag attention weights BD[p,(g,thi,m)] (GpSimd) ----
            BD = small_pool.tile([128, G, THI, N32], F32)      # [128,1024]
            a_b = attn_v[:, :, :, None].broadcast_to([128, G, THI, N32])
            m_b = mask_sb[:][:, None, None, :].broadcast_to([128, G, THI, N32])
            nc.gpsimd.tensor_mul(BD[:], a_b, m_b)

            # ---- P4: out = BD^T @ (cb slice), 4 groups col-tiled across PE
            # quadrants.  group g = 4*gh + j -> PSUM partitions [32j,32j+32),
            # free cols [64*gh, 64*gh+64).  One PSUM start per quadrant (the
            # start zeroes the whole 2KB bank in the written partitions);
            # everything else accumulates. ----
            ops = ops_pool.tile([128, (G // 4) * D], F32)      # [128,128] psum
            ops_v = ops[:].rearrange("p (gh d) -> p gh d", gh=G // 4)
            for thi in range(THI):
                for qb in range(0, G, 4):
                    gh = qb // 4
                    for j in range(4):
                        g = qb + j
                        nc.tensor.matmul(
                            ops_v[32 * j:32 * j + 32, gh],
                            BD[:, g, thi], cb_v[:, g, thi],
                            start=(thi == 0 and qb == 0),
                            stop=(thi == THI - 1 and qb == G - 4),
                            tile_position=(0, 32 * j),
                            skip_group_check=True,
                        )

            outsb = outsb_pool.tile([128, (G // 4) * D], F32)  # [128,128]
            nc.scalar.copy(outsb[:], ops[:])
            src = outsb[:].rearrange("p (gh d) -> p gh d", gh=G // 4)
            nc.scalar.dma_start(o_ap[ts], src)

    nc.compile()
    return nc


def make_consts():
    p = np.arange(128)
    mask = (p[:, None] // TLO == np.arange(N32)[None, :]).astype(np.float32)
    return mask, np.ascontiguousarray(mask.T)


_CACHE = {}


def _get_program(npc):
    if npc not in _CACHE:
        _CACHE[npc] = build_program(npc=npc)
    return _CACHE[npc]


def kernel(c_ft: np.ndarray, ft: np.ndarray) -> np.ndarray:
    c_ft = np.ascontiguousarray(np.asarray(c_ft), dtype=np.float32)
    ft = np.ascontiguousarray(np.asarray(ft), dtype=np.float32)
    n = c_ft.shape[0]
    total = N_CORES * NPC
    if n < total:
        c_pad = np.zeros((total, T, D), np.float32)
        c_pad[:n] = c_ft
        f_pad = np.zeros((total, D), np.float32)
        f_pad[:n] = ft
    else:
        c_pad, f_pad = c_ft, ft

    mask, maskT = make_consts()
    nc = _get_program(NPC)
    in_maps = [
        {
            "c_ft": c_pad[i * NPC:(i + 1) * NPC],
            "ft": f_pad[i * NPC:(i + 1) * NPC],
            "mask32": mask,
            "m32T": maskT,
        }
        for i in range(N_CORES)
    ]
    res = run_bass_kernel_spmd(
        nc, in_maps, core_ids=list(range(N_CORES)),
        trace=bool(int(os.environ.get("KERNEL_TRACE", "0"))),
    )
    out = np.concatenate([r["out"] for r in res.results], axis=0)
    if bool(int(os.environ.get("KERNEL_TRACE", "0"))):
        kernel.last_exec_time_ns = res.exec_time_ns
        kernel.last_results = res
    return out[:n]
